# revision 19
# baseline (speedup 1.0000x reference)
"""NeuralODE Euler kernel v5: baseline S=4 structure + fp8 DoubleRow.

Measured: 393.3us HW (v3 baseline 424.4us), rel err 1.239e-2 (gate 2e-2).
v3 was PE+ACT co-saturated: 32 bf16 matmuls/step (2032ns period). v5 keeps
the proven 4-stream FD=128 structure and relu split (h1->ACT, h2->DVE) but:
- L2 and W31 become ONE fp8 e4m3 DoubleRow matmul per (stream, mc):
  K=256 in a single MM at the same issue rate as bf16 K=128 (measured
  109ns at N=256, 55ns at N=128; LDWEIGHTS hidden). PE/step: 2032->~900.
  Steady state 1893ns/step, ACT-saturated (4x473ns incl sems) with DVE at
  ~1858; this is ~the floor: 2048 fp32 PSUM elems/step must cross the only
  two PSUM read ports (ACT (352+FD)/1.2, DVE (184+FD)/0.96 per instr), and
  every fewer-instruction structure (S=2/3, merged pairs) is bound by the
  per-stream chain relu->L2->relu->W31 at 1950-2470ns (S=3 measured 2320).
- h2 shipped fp8 (128KB/step/core, was 256KB bf16).
- warmup matmuls use dep-free scratch tiles so the HAM p-state burst
  overlaps the input DMAs; y0/w1 DMAs issue first (init needs only those;
  DMA data latency ~2.5us). nwarm=16 x 213ns spans the 3.4us HAM window so
  K=8/8 fires BEFORE step 0 (shorter warmups left up to 9 cold 2287ns
  steps; gpsimd-issued output DMAs contend with DVE's shared SBUF port
  and cost +18us - do not use).

Scaling (static, folded). u' = a*u in PSUM, a = s31*d:
  h1' = fp8(relu(u' * (b/a)))            ACT scale imm
  ps2' = W2q^T h1', W2q = fp8(s2*W2)     DR matmul (= s2*b*ps2)
  h2' = fp8(max(ps2' * d/(s2*b), 0))     DVE op0=mult op1=max (= d*h2)
  u' += W31q^T h2', W31q = fp8(s31*W31)  DR accumulate (= a*(W31^T h2))
Host: h2 = h2'/d; dy = (dt W3)^T h2 + dt b3; fp32 cumsum.
Quantized-pipeline rel err vs fp32 reference: 1.24e-2 (gate 2e-2, measured
on HW); W2 quantization dominates (1.09e-2).
"""
import numpy as np

import concourse.bass as bass
import concourse.tile as tile
from concourse import bacc, mybir
from concourse.bass_utils import run_bass_kernel_spmd

F32 = mybir.dt.float32
BF16 = mybir.dt.bfloat16
FP8 = mybir.dt.float8e4
RELU = mybir.ActivationFunctionType.Relu
mult = mybir.AluOpType.mult
mx = mybir.AluOpType.max
DR = mybir.MatmulPerfMode.DoubleRow

B, D, H, T = 4096, 64, 256, 200
NCORES = 8
BL = B // NCORES          # 512
NS = 4
FD = BL // NS             # 128
FDS = [FD] * NS
OFF = [s * FD for s in range(NS)]

H1AMAX_EST = 5.5          # sim-measured 4.39 (+margin); device clips at 240
H2AMAX_EST = 7.5          # sim-measured 6.12

_cache = {}


def build(nsteps: int, h1scale: float, h2scale: float, nwarm: int = 16,
          has_b1=False, has_b2=False, has_b3=False):
    nc = bacc.Bacc("TRN2", target_bir_lowering=False, debug=False)
    y0_d = nc.dram_tensor("y0T", [D, BL], BF16, kind="ExternalInput")
    w1_d = nc.dram_tensor("w1a", [D, 2, 128], BF16, kind="ExternalInput")
    w2_d = nc.dram_tensor("w2q", [128, 2, 2, 128], FP8, kind="ExternalInput")
    w31_d = nc.dram_tensor("w31q", [128, 2, 2, 128], FP8,
                           kind="ExternalInput")
    b1_d = nc.dram_tensor("b1r", [1, 2, 128], BF16, kind="ExternalInput")
    b2_d = nc.dram_tensor("b2r", [1, 2, 128], BF16, kind="ExternalInput")
    c1_d = nc.dram_tensor("c1r", [1, 2, 128], BF16, kind="ExternalInput")
    out_d = nc.dram_tensor("out", [nsteps, 128, 2, BL], FP8,
                           kind="ExternalOutput")

    with tile.TileContext(nc) as tc:
        with tc.tile_pool(name="wpool", bufs=1) as wp, \
             tc.tile_pool(name="hpool", bufs=3) as hp, \
             tc.tile_pool(name="stpool", bufs=3) as stp, \
             tc.tile_pool(name="up", bufs=1, space="PSUM") as up, \
             tc.tile_pool(name="ps2p", bufs=1, space="PSUM") as pp:

            w1 = wp.tile([D, 2, 128], BF16)
            w2 = wp.tile([128, 2, 2, 128], FP8)
            w31 = wp.tile([128, 2, 2, 128], FP8)
            y0s = wp.tile([D, BL], BF16)
            wsc = wp.tile([128, 512], BF16)   # dep-free warmup scratch
            nc.vector.memset(wsc[:], 0.125)
            # y0+w1 first: init matmuls need only these; w2/w31 are not
            # read until the first step's L2/W31 (~2us later)
            nc.sync.dma_start(y0s[:], y0_d.ap())
            nc.sync.dma_start(w1[:], w1_d.ap())
            nc.sync.dma_start(w2[:], w2_d.ap())
            nc.sync.dma_start(w31[:], w31_d.ap())
            if has_b1 or has_b2 or has_b3:
                ones = wp.tile([1, FD], BF16)
                nc.vector.memset(ones[:], 1.0)
            if has_b1:
                b1r = wp.tile([1, 2, 128], BF16)
                nc.sync.dma_start(b1r[:], b1_d.ap())
            if has_b2:
                b2r = wp.tile([1, 2, 128], BF16)
                nc.sync.dma_start(b2r[:], b2_d.ap())
            if has_b3:
                c1r = wp.tile([1, 2, 128], BF16)
                nc.sync.dma_start(c1r[:], c1_d.ap())

            us = [up.tile([128, 2, FD], F32, tag=f"u{s}", name=f"u{s}")
                  for s in range(NS)]
            ps2s = [pp.tile([128, 2, FD], F32, tag=f"ps2{s}", name=f"ps2{s}")
                    for s in range(NS)]

            # HAM p-state warmup on scratch (no DMA deps -> overlaps input
            # DMAs); ps2 banks are rewritten fresh each step (start=True)
            for w in range(nwarm):
                nc.tensor.matmul(
                    ps2s[w % NS][:, :, :].rearrange("p a b -> p (a b)"),
                    wsc[:, 0:128], wsc[:, 0:256], start=True, stop=True,
                    skip_group_check=True)

            # init u' = a*(W1^T y0) (+ a*b1); first matmul per bank starts
            for s in range(NS):
                cs = slice(OFF[s], OFF[s] + FDS[s])
                for mc in range(2):
                    nc.tensor.matmul(us[s][:, mc, 0:FDS[s]], w1[:, mc, :],
                                     y0s[:, cs], start=(mc == 0),
                                     stop=(mc == 1) and not has_b1,
                                     skip_group_check=True)
                    if has_b1:
                        nc.tensor.matmul(us[s][:, mc, 0:FDS[s]],
                                         b1r[:, mc, :], ones[:, 0:FDS[s]],
                                         start=False, stop=(mc == 1),
                                         skip_group_check=True)

            out_ap = out_d.ap()

            for i in range(nsteps):
                h2a = stp.tile([128, 2, BL], FP8, tag="h2all", name="h2all")
                order = [(i - so) % NS for so in range(NS)]
                h1s = {}
                for s in order:
                    fd = FDS[s]
                    # ---- h1' = fp8(relu(u' * b/a)) on ACT ----
                    h1s[s] = hp.tile([128, 2, fd], FP8, tag=f"h1s{s}",
                                     name="h1")
                    nc.scalar.activation(h1s[s][:], us[s][:, :, 0:fd], RELU,
                                         scale=h1scale)
                for s in order:
                    fd = FDS[s]
                    # ---- layer 2: DoubleRow matmuls; mc1 split with a
                    # small N=16 tail so ps2-complete lands ~20ns earlier
                    # (chain: completion = issue(total-lastN) + dur(lastN))
                    nc.tensor.matmul(ps2s[s][:, 0, 0:fd],
                                     w2[:, :, 0, :], h1s[s][:],
                                     start=True, stop=not has_b2,
                                     perf_mode=DR, skip_group_check=True)
                    nc.tensor.matmul(ps2s[s][:, 1, 0:112],
                                     w2[:, :, 1, :], h1s[s][:, :, 0:112],
                                     start=True, stop=not has_b2,
                                     perf_mode=DR, skip_group_check=True)
                    nc.tensor.matmul(ps2s[s][:, 1, 112:fd],
                                     w2[:, :, 1, :], h1s[s][:, :, 112:fd],
                                     start=True, stop=not has_b2,
                                     perf_mode=DR, skip_group_check=True)
                    for mc in []:
                        if has_b2:
                            nc.tensor.matmul(ps2s[s][:, mc, 0:fd],
                                             b2r[:, mc, :], ones[:, 0:fd],
                                             start=False, stop=True,
                                             skip_group_check=True)
                for s in order:
                    fd = FDS[s]
                    cs = slice(OFF[s], OFF[s] + fd)
                    # ---- h2' = fp8(max(ps2' * d/(s2 b), 0)) on DVE ----
                    nc.vector.tensor_scalar(h2a[:, :, cs],
                                            ps2s[s][:, :, 0:fd],
                                            h2scale, 0.0, op0=mult, op1=mx)
                if i < 4:
                    # scratch matmuls into the (dead) ps2 banks: raise
                    # early-step PE duty to ~97% so the free-running HAM
                    # window fires K=8/8 by ~14us worst-case regardless of
                    # phase (else up to 18 cold 2287ns steps). ps2 is dead
                    # here: DVE consumed it, next L2 re-starts the bank.
                    for w in range(4):
                        nc.tensor.matmul(ps2s[order[w]][:, 0, :],
                                         wsc[:, 0:128], wsc[:, 0:128],
                                         start=True, stop=True,
                                         skip_group_check=True)
                for s in order:
                    fd = FDS[s]
                    cs = slice(OFF[s], OFF[s] + fd)
                    co = OFF[s]
                    # ---- u' += W31q^T h2' (DR accumulate, split tail) ----
                    nc.tensor.matmul(us[s][:, 0, 0:fd],
                                     w31[:, :, 0, :],
                                     h2a[:, :, cs], start=False,
                                     stop=False,
                                     perf_mode=DR, skip_group_check=True)
                    nc.tensor.matmul(us[s][:, 1, 0:112],
                                     w31[:, :, 1, :],
                                     h2a[:, :, co:co + 112], start=False,
                                     stop=False,
                                     perf_mode=DR, skip_group_check=True)
                    nc.tensor.matmul(us[s][:, 1, 112:fd],
                                     w31[:, :, 1, :],
                                     h2a[:, :, co + 112:co + fd],
                                     start=False,
                                     stop=not has_b3,
                                     perf_mode=DR, skip_group_check=True)
                    for mc in []:
                        if has_b3:
                            nc.tensor.matmul(us[s][:, mc, 0:fd],
                                             c1r[:, mc, :], ones[:, 0:fd],
                                             start=False, stop=(mc == 1),
                                             skip_group_check=True)
                # ship h2' to HBM; host divides by d and projects via dt*W3
                nc.sync.dma_start(out_ap[i], h2a[:])
    nc.compile()
    return nc


def _prep_inputs(y0, t, W1, b1, W2, b2, W3, b3):
    import ml_dtypes
    bf16 = ml_dtypes.bfloat16
    e4 = ml_dtypes.float8_e4m3fn
    dt64 = np.float64(t[1]) - np.float64(t[0])
    W31 = (dt64 * (W3.astype(np.float64) @ W1.astype(np.float64))).astype(
        np.float32)
    c1f = (dt64 * (b3.astype(np.float64) @ W1.astype(np.float64))).astype(
        np.float32)
    W3d = (dt64 * W3.astype(np.float64)).astype(np.float32)
    b3d = (dt64 * b3.astype(np.float64)).astype(np.float32)

    s2 = 240.0 / max(float(np.abs(W2).max()), 1e-30)
    s31 = 240.0 / max(float(np.abs(W31).max()), 1e-30)
    bsc = 224.0 / H1AMAX_EST
    dsc = 224.0 / H2AMAX_EST
    a = s31 * dsc

    def q8(x, s):
        return np.clip(x.astype(np.float64) * s, -240, 240).astype(
            np.float32).astype(e4)

    w2q = np.ascontiguousarray(
        q8(W2, s2).reshape(2, 128, 2, 128).transpose(1, 0, 2, 3))
    w31q = np.ascontiguousarray(
        q8(W31, s31).reshape(2, 128, 2, 128).transpose(1, 0, 2, 3))
    w1a = np.ascontiguousarray(
        (a * W1.astype(np.float64)).astype(np.float32).reshape(
            D, 2, 128)).astype(bf16)
    b1r = np.ascontiguousarray(
        (a * b1.astype(np.float64)).astype(np.float32).reshape(
            1, 2, 128)).astype(bf16)
    b2r = np.ascontiguousarray(
        (s2 * bsc * b2.astype(np.float64)).astype(np.float32).reshape(
            1, 2, 128)).astype(bf16)
    c1r = np.ascontiguousarray(
        (a * c1f.astype(np.float64)).astype(np.float32).reshape(
            1, 2, 128)).astype(bf16)

    in_maps = []
    for c in range(NCORES):
        y0T = np.ascontiguousarray(y0[c * BL:(c + 1) * BL].T).astype(bf16)
        in_maps.append({"y0T": y0T, "w1a": w1a, "w2q": w2q, "w31q": w31q,
                        "b1r": b1r, "b2r": b2r, "c1r": c1r})
    scales = dict(h1scale=float(bsc / a), h2scale=float(dsc / (s2 * bsc)),
                  d=dsc)
    return in_maps, W3d, b3d, scales


def kernel(y0, t, W1, b1, W2, b2, W3, b3, nwarm: int = 16, **run_kwargs):
    import ml_dtypes
    nsteps = int(t.shape[0]) - 1
    has_b1 = bool(np.any(b1)); has_b2 = bool(np.any(b2))
    has_b3 = bool(np.any(b3))
    in_maps, W3d, b3d, sc = _prep_inputs(y0, t, W1, b1, W2, b2, W3, b3)
    key = (nsteps, nwarm, has_b1, has_b2, has_b3,
           round(sc["h1scale"], 14), round(sc["h2scale"], 14))
    if key not in _cache:
        _cache[key] = build(nsteps, sc["h1scale"], sc["h2scale"], nwarm,
                            has_b1, has_b2, has_b3)
    nc = _cache[key]
    res = run_bass_kernel_spmd(nc, in_maps, core_ids=list(range(NCORES)),
                               **run_kwargs)
    inv_d = np.float32(1.0 / sc["d"])
    parts = []
    for c in range(NCORES):
        h2 = res.results[c]["out"]        # [nsteps, 128, 2, BL] fp8
        if h2.dtype != ml_dtypes.float8_e4m3fn:
            h2 = h2.view(ml_dtypes.float8_e4m3fn)
        nst = h2.shape[0]
        hh = h2.astype(np.float32).transpose(0, 2, 1, 3).reshape(nst, H, BL)
        hh *= inv_d
        dy = np.tensordot(hh, W3d, axes=([1], [0]))       # [nsteps, BL, D]
        dy += b3d
        dy = np.ascontiguousarray(dy.transpose(1, 0, 2))  # [BL, nsteps, D]
        yb = y0[c * BL:(c + 1) * BL].astype(np.float32)
        ys = yb[:, None, :] + np.cumsum(dy, axis=1, dtype=np.float32)
        parts.append(np.concatenate([yb[:, None, :], ys], axis=1))
    return np.concatenate(parts, axis=0).astype(np.float32)


# revision 20
# speedup vs baseline: 1.1108x; 1.1108x over previous
"""NeuralODE Euler kernel v5: baseline S=4 structure + fp8 DoubleRow.

Measured: 393.3us HW (v3 baseline 424.4us), rel err 1.239e-2 (gate 2e-2).
v3 was PE+ACT co-saturated: 32 bf16 matmuls/step (2032ns period). v5 keeps
the proven 4-stream FD=128 structure and relu split (h1->ACT, h2->DVE) but:
- L2 and W31 become ONE fp8 e4m3 DoubleRow matmul per (stream, mc):
  K=256 in a single MM at the same issue rate as bf16 K=128 (measured
  109ns at N=256, 55ns at N=128; LDWEIGHTS hidden). PE/step: 2032->~900.
  Steady state 1893ns/step, ACT-saturated (4x473ns incl sems) with DVE at
  ~1858; this is ~the floor: 2048 fp32 PSUM elems/step must cross the only
  two PSUM read ports (ACT (352+FD)/1.2, DVE (184+FD)/0.96 per instr), and
  every fewer-instruction structure (S=2/3, merged pairs) is bound by the
  per-stream chain relu->L2->relu->W31 at 1950-2470ns (S=3 measured 2320).
- h2 shipped fp8 (128KB/step/core, was 256KB bf16).
- warmup matmuls use dep-free scratch tiles so the HAM p-state burst
  overlaps the input DMAs; y0/w1 DMAs issue first (init needs only those;
  DMA data latency ~2.5us). nwarm=16 x 213ns spans the 3.4us HAM window so
  K=8/8 fires BEFORE step 0 (shorter warmups left up to 9 cold 2287ns
  steps; gpsimd-issued output DMAs contend with DVE's shared SBUF port
  and cost +18us - do not use).

Scaling (static, folded). u' = a*u in PSUM, a = s31*d:
  h1' = fp8(relu(u' * (b/a)))            ACT scale imm
  ps2' = W2q^T h1', W2q = fp8(s2*W2)     DR matmul (= s2*b*ps2)
  h2' = fp8(max(ps2' * d/(s2*b), 0))     DVE op0=mult op1=max (= d*h2)
  u' += W31q^T h2', W31q = fp8(s31*W31)  DR accumulate (= a*(W31^T h2))
Host: h2 = h2'/d; dy = (dt W3)^T h2 + dt b3; fp32 cumsum.
Quantized-pipeline rel err vs fp32 reference: 1.24e-2 (gate 2e-2, measured
on HW); W2 quantization dominates (1.09e-2).
"""
import numpy as np

import concourse.bass as bass
import concourse.tile as tile
from concourse import bacc, mybir
from concourse.bass_utils import run_bass_kernel_spmd

F32 = mybir.dt.float32
BF16 = mybir.dt.bfloat16
FP8 = mybir.dt.float8e4
RELU = mybir.ActivationFunctionType.Relu
mult = mybir.AluOpType.mult
mx = mybir.AluOpType.max
DR = mybir.MatmulPerfMode.DoubleRow

B, D, H, T = 4096, 64, 256, 200
NCORES = 8
BL = B // NCORES          # 512
NS = 4
FD = BL // NS             # 128
FDS = [FD] * NS
OFF = [s * FD for s in range(NS)]

H1AMAX_EST = 5.5          # sim-measured 4.39 (+margin); device clips at 240
H2AMAX_EST = 7.5          # sim-measured 6.12

_cache = {}


def build(nsteps: int, h1scale: float, h2scale: float, nwarm: int = 16,
          has_b1=False, has_b2=False, has_b3=False):
    nc = bacc.Bacc("TRN2", target_bir_lowering=False, debug=False)
    y0_d = nc.dram_tensor("y0T", [D, BL], BF16, kind="ExternalInput")
    w1_d = nc.dram_tensor("w1a", [D, 2, 128], BF16, kind="ExternalInput")
    w2_d = nc.dram_tensor("w2q", [128, 2, 2, 128], FP8, kind="ExternalInput")
    w31_d = nc.dram_tensor("w31q", [128, 2, 2, 128], FP8,
                           kind="ExternalInput")
    b1_d = nc.dram_tensor("b1r", [1, 2, 128], BF16, kind="ExternalInput")
    b2_d = nc.dram_tensor("b2r", [1, 2, 128], BF16, kind="ExternalInput")
    c1_d = nc.dram_tensor("c1r", [1, 2, 128], BF16, kind="ExternalInput")
    out_d = nc.dram_tensor("out", [nsteps, 128, 2, BL], FP8,
                           kind="ExternalOutput")

    with tile.TileContext(nc) as tc:
        with tc.tile_pool(name="wpool", bufs=1) as wp, \
             tc.tile_pool(name="hpool", bufs=3) as hp, \
             tc.tile_pool(name="stpool", bufs=3) as stp, \
             tc.tile_pool(name="up", bufs=1, space="PSUM") as up, \
             tc.tile_pool(name="ps2p", bufs=1, space="PSUM") as pp:

            w1 = wp.tile([D, 2, 128], BF16)
            w2 = wp.tile([128, 2, 2, 128], FP8)
            w31 = wp.tile([128, 2, 2, 128], FP8)
            y0s = wp.tile([D, BL], BF16)
            wsc = wp.tile([128, 512], BF16)   # dep-free warmup scratch
            nc.vector.memset(wsc[:], 0.125)
            # y0+w1 first: init matmuls need only these; w2/w31 are not
            # read until the first step's L2/W31 (~2us later)
            nc.sync.dma_start(y0s[:], y0_d.ap())
            nc.sync.dma_start(w1[:], w1_d.ap())
            nc.sync.dma_start(w2[:], w2_d.ap())
            nc.sync.dma_start(w31[:], w31_d.ap())
            if has_b1 or has_b2 or has_b3:
                ones = wp.tile([1, FD], BF16)
                nc.vector.memset(ones[:], 1.0)
            if has_b1:
                b1r = wp.tile([1, 2, 128], BF16)
                nc.sync.dma_start(b1r[:], b1_d.ap())
            if has_b2:
                b2r = wp.tile([1, 2, 128], BF16)
                nc.sync.dma_start(b2r[:], b2_d.ap())
            if has_b3:
                c1r = wp.tile([1, 2, 128], BF16)
                nc.sync.dma_start(c1r[:], c1_d.ap())

            us = [up.tile([128, 2, FD], F32, tag=f"u{s}", name=f"u{s}")
                  for s in range(NS)]
            ps2s = [pp.tile([128, 2, FD], F32, tag=f"ps2{s}", name=f"ps2{s}")
                    for s in range(NS)]

            # HAM p-state warmup on scratch (no DMA deps -> overlaps input
            # DMAs); ps2 banks are rewritten fresh each step (start=True)
            for w in range(nwarm):
                nc.tensor.matmul(
                    ps2s[w % NS][:, :, :].rearrange("p a b -> p (a b)"),
                    wsc[:, 0:128], wsc[:, 0:256], start=True, stop=True,
                    skip_group_check=True)

            # init u' = a*(W1^T y0) (+ a*b1); first matmul per bank starts
            for s in range(NS):
                cs = slice(OFF[s], OFF[s] + FDS[s])
                for mc in range(2):
                    nc.tensor.matmul(us[s][:, mc, 0:FDS[s]], w1[:, mc, :],
                                     y0s[:, cs], start=(mc == 0),
                                     stop=(mc == 1) and not has_b1,
                                     skip_group_check=True)
                    if has_b1:
                        nc.tensor.matmul(us[s][:, mc, 0:FDS[s]],
                                         b1r[:, mc, :], ones[:, 0:FDS[s]],
                                         start=False, stop=(mc == 1),
                                         skip_group_check=True)

            out_ap = out_d.ap()

            for i in range(nsteps):
                h2a = stp.tile([128, 2, BL], FP8, tag="h2all", name="h2all")
                order = [(i - so) % NS for so in range(NS)]
                h1s = {}
                for s in order:
                    fd = FDS[s]
                    # ---- h1' = fp8(relu(u' * b/a)) on ACT ----
                    h1s[s] = hp.tile([128, 2, fd], FP8, tag=f"h1s{s}",
                                     name="h1")
                    nc.scalar.activation(h1s[s][:], us[s][:, :, 0:fd], RELU,
                                         scale=h1scale)
                for s in order:
                    fd = FDS[s]
                    # ---- layer 2: one DoubleRow matmul per mc ----
                    for mc in range(2):
                        nc.tensor.matmul(ps2s[s][:, mc, 0:fd],
                                         w2[:, :, mc, :], h1s[s][:],
                                         start=True, stop=not has_b2,
                                         perf_mode=DR, skip_group_check=True)
                        if has_b2:
                            nc.tensor.matmul(ps2s[s][:, mc, 0:fd],
                                             b2r[:, mc, :], ones[:, 0:fd],
                                             start=False, stop=True,
                                             skip_group_check=True)
                for s in order:
                    fd = FDS[s]
                    cs = slice(OFF[s], OFF[s] + fd)
                    # ---- h2' = fp8(max(ps2' * d/(s2 b), 0)) on DVE ----
                    nc.vector.tensor_scalar(h2a[:, :, cs],
                                            ps2s[s][:, :, 0:fd],
                                            h2scale, 0.0, op0=mult, op1=mx)
                if i < 4:
                    # scratch matmuls into the (dead) ps2 banks: raise
                    # early-step PE duty to ~97% so the free-running HAM
                    # window fires K=8/8 by ~14us worst-case regardless of
                    # phase (else up to 18 cold 2287ns steps). ps2 is dead
                    # here: DVE consumed it, next L2 re-starts the bank.
                    for w in range(4):
                        nc.tensor.matmul(ps2s[order[w]][:, 0, :],
                                         wsc[:, 0:128], wsc[:, 0:128],
                                         start=True, stop=True,
                                         skip_group_check=True)
                for s in order:
                    fd = FDS[s]
                    cs = slice(OFF[s], OFF[s] + fd)
                    # ---- u' += W31q^T h2' (DR accumulate) ----
                    for mc in range(2):
                        nc.tensor.matmul(us[s][:, mc, 0:fd],
                                         w31[:, :, mc, :],
                                         h2a[:, :, cs], start=False,
                                         stop=(mc == 1) and not has_b3,
                                         perf_mode=DR, skip_group_check=True)
                        if has_b3:
                            nc.tensor.matmul(us[s][:, mc, 0:fd],
                                             c1r[:, mc, :], ones[:, 0:fd],
                                             start=False, stop=(mc == 1),
                                             skip_group_check=True)
                # ship h2' to HBM; host divides by d and projects via dt*W3
                nc.sync.dma_start(out_ap[i], h2a[:])
    nc.compile()
    return nc


def _prep_inputs(y0, t, W1, b1, W2, b2, W3, b3):
    import ml_dtypes
    bf16 = ml_dtypes.bfloat16
    e4 = ml_dtypes.float8_e4m3fn
    dt64 = np.float64(t[1]) - np.float64(t[0])
    W31 = (dt64 * (W3.astype(np.float64) @ W1.astype(np.float64))).astype(
        np.float32)
    c1f = (dt64 * (b3.astype(np.float64) @ W1.astype(np.float64))).astype(
        np.float32)
    W3d = (dt64 * W3.astype(np.float64)).astype(np.float32)
    b3d = (dt64 * b3.astype(np.float64)).astype(np.float32)

    s2 = 240.0 / max(float(np.abs(W2).max()), 1e-30)
    s31 = 240.0 / max(float(np.abs(W31).max()), 1e-30)
    bsc = 224.0 / H1AMAX_EST
    dsc = 224.0 / H2AMAX_EST
    a = s31 * dsc

    def q8(x, s):
        return np.clip(x.astype(np.float64) * s, -240, 240).astype(
            np.float32).astype(e4)

    w2q = np.ascontiguousarray(
        q8(W2, s2).reshape(2, 128, 2, 128).transpose(1, 0, 2, 3))
    w31q = np.ascontiguousarray(
        q8(W31, s31).reshape(2, 128, 2, 128).transpose(1, 0, 2, 3))
    w1a = np.ascontiguousarray(
        (a * W1.astype(np.float64)).astype(np.float32).reshape(
            D, 2, 128)).astype(bf16)
    b1r = np.ascontiguousarray(
        (a * b1.astype(np.float64)).astype(np.float32).reshape(
            1, 2, 128)).astype(bf16)
    b2r = np.ascontiguousarray(
        (s2 * bsc * b2.astype(np.float64)).astype(np.float32).reshape(
            1, 2, 128)).astype(bf16)
    c1r = np.ascontiguousarray(
        (a * c1f.astype(np.float64)).astype(np.float32).reshape(
            1, 2, 128)).astype(bf16)

    in_maps = []
    for c in range(NCORES):
        y0T = np.ascontiguousarray(y0[c * BL:(c + 1) * BL].T).astype(bf16)
        in_maps.append({"y0T": y0T, "w1a": w1a, "w2q": w2q, "w31q": w31q,
                        "b1r": b1r, "b2r": b2r, "c1r": c1r})
    scales = dict(h1scale=float(bsc / a), h2scale=float(dsc / (s2 * bsc)),
                  d=dsc)
    return in_maps, W3d, b3d, scales


def kernel(y0, t, W1, b1, W2, b2, W3, b3, nwarm: int = 16, **run_kwargs):
    import ml_dtypes
    nsteps = int(t.shape[0]) - 1
    has_b1 = bool(np.any(b1)); has_b2 = bool(np.any(b2))
    has_b3 = bool(np.any(b3))
    in_maps, W3d, b3d, sc = _prep_inputs(y0, t, W1, b1, W2, b2, W3, b3)
    key = (nsteps, nwarm, has_b1, has_b2, has_b3,
           round(sc["h1scale"], 14), round(sc["h2scale"], 14))
    if key not in _cache:
        _cache[key] = build(nsteps, sc["h1scale"], sc["h2scale"], nwarm,
                            has_b1, has_b2, has_b3)
    nc = _cache[key]
    res = run_bass_kernel_spmd(nc, in_maps, core_ids=list(range(NCORES)),
                               **run_kwargs)
    inv_d = np.float32(1.0 / sc["d"])
    parts = []
    for c in range(NCORES):
        h2 = res.results[c]["out"]        # [nsteps, 128, 2, BL] fp8
        if h2.dtype != ml_dtypes.float8_e4m3fn:
            h2 = h2.view(ml_dtypes.float8_e4m3fn)
        nst = h2.shape[0]
        hh = h2.astype(np.float32).transpose(0, 2, 1, 3).reshape(nst, H, BL)
        hh *= inv_d
        dy = np.tensordot(hh, W3d, axes=([1], [0]))       # [nsteps, BL, D]
        dy += b3d
        dy = np.ascontiguousarray(dy.transpose(1, 0, 2))  # [BL, nsteps, D]
        yb = y0[c * BL:(c + 1) * BL].astype(np.float32)
        ys = yb[:, None, :] + np.cumsum(dy, axis=1, dtype=np.float32)
        parts.append(np.concatenate([yb[:, None, :], ys], axis=1))
    return np.concatenate(parts, axis=0).astype(np.float32)


# revision 21
# speedup vs baseline: 1.1110x; 1.0002x over previous
"""NeuralODE Euler kernel v5: baseline S=4 structure + fp8 DoubleRow.

Measured: 393.6-393.9us HW (v3 baseline 424.4us), rel err 1.239e-2.
v3 was PE+ACT co-saturated: 32 bf16 matmuls/step (2032ns period). v5 keeps
the proven 4-stream FD=128 structure and relu split (h1->ACT, h2->DVE) but:
- L2 and W31 become ONE fp8 e4m3 DoubleRow matmul per (stream, mc):
  K=256 in a single MM at the same issue rate as bf16 K=128 (measured
  109ns at N=256, 55ns at N=128; LDWEIGHTS hidden). PE/step: 2032->~900.
  Steady state 1893ns/step, ACT-saturated (4x473ns incl sems) with DVE at
  ~1858; this is ~the floor: 2048 fp32 PSUM elems/step must cross the only
  two PSUM read ports (ACT (352+FD)/1.2, DVE (184+FD)/0.96 per instr), and
  every fewer-instruction structure (S=2/3, merged pairs) is bound by the
  per-stream chain relu->L2->relu->W31 at 1950-2470ns (S=3 measured 2320).
- h2 shipped fp8 (128KB/step/core, was 256KB bf16).
- warmup matmuls use dep-free scratch tiles so the HAM p-state burst
  overlaps the input DMAs; y0/w1 DMAs issue first (init needs only those;
  DMA data latency ~2.5us). nwarm=16 x 213ns spans the 3.4us HAM window so
  K=8/8 fires BEFORE step 0; 4 scratch matmuls into the dead ps2 banks
  during steps i<4 keep early PE duty ~97% so firing is deterministic
  (~13us) regardless of the free-running HAM window phase (without them,
  up to 18 cold 2287ns steps when unlucky; gpsimd-issued output DMAs
  contend with DVE's shared SBUF port and cost +18us - do not use).

Scaling (static, folded). u' = a*u in PSUM, a = s31*d:
  h1' = fp8(relu(u' * (b/a)))            ACT scale imm
  ps2' = W2q^T h1', W2q = fp8(s2*W2)     DR matmul (= s2*b*ps2)
  h2' = fp8(max(ps2' * d/(s2*b), 0))     DVE op0=mult op1=max (= d*h2)
  u' += W31q^T h2', W31q = fp8(s31*W31)  DR accumulate (= a*(W31^T h2))
Host: h2 = h2'/d; dy = (dt W3)^T h2 + dt b3; fp32 cumsum.
Quantized-pipeline rel err vs fp32 reference: 1.24e-2 (gate 2e-2, measured
on HW); W2 quantization dominates (1.09e-2).
"""
import numpy as np

import concourse.bass as bass
import concourse.tile as tile
from concourse import bacc, mybir
from concourse.bass_utils import run_bass_kernel_spmd

F32 = mybir.dt.float32
BF16 = mybir.dt.bfloat16
FP8 = mybir.dt.float8e4
RELU = mybir.ActivationFunctionType.Relu
mult = mybir.AluOpType.mult
mx = mybir.AluOpType.max
DR = mybir.MatmulPerfMode.DoubleRow

B, D, H, T = 4096, 64, 256, 200
NCORES = 8
BL = B // NCORES          # 512
NS = 4
FD = BL // NS             # 128
FDS = [FD] * NS
OFF = [s * FD for s in range(NS)]

H1AMAX_EST = 5.5          # sim-measured 4.39 (+margin); device clips at 240
H2AMAX_EST = 7.5          # sim-measured 6.12

_cache = {}


def build(nsteps: int, h1scale: float, h2scale: float, nwarm: int = 16,
          has_b1=False, has_b2=False, has_b3=False):
    nc = bacc.Bacc("TRN2", target_bir_lowering=False, debug=False)
    y0_d = nc.dram_tensor("y0T", [D, BL], BF16, kind="ExternalInput")
    w1_d = nc.dram_tensor("w1a", [D, 2, 128], BF16, kind="ExternalInput")
    w2_d = nc.dram_tensor("w2q", [128, 2, 2, 128], FP8, kind="ExternalInput")
    w31_d = nc.dram_tensor("w31q", [128, 2, 2, 128], FP8,
                           kind="ExternalInput")
    b1_d = nc.dram_tensor("b1r", [1, 2, 128], BF16, kind="ExternalInput")
    b2_d = nc.dram_tensor("b2r", [1, 2, 128], BF16, kind="ExternalInput")
    c1_d = nc.dram_tensor("c1r", [1, 2, 128], BF16, kind="ExternalInput")
    out_d = nc.dram_tensor("out", [nsteps, 128, 2, BL], FP8,
                           kind="ExternalOutput")

    with tile.TileContext(nc) as tc:
        with tc.tile_pool(name="wpool", bufs=1) as wp, \
             tc.tile_pool(name="hpool", bufs=3) as hp, \
             tc.tile_pool(name="stpool", bufs=3) as stp, \
             tc.tile_pool(name="up", bufs=1, space="PSUM") as up, \
             tc.tile_pool(name="ps2p", bufs=1, space="PSUM") as pp:

            w1 = wp.tile([D, 2, 128], BF16)
            w2 = wp.tile([128, 2, 2, 128], FP8)
            w31 = wp.tile([128, 2, 2, 128], FP8)
            y0s = wp.tile([D, BL], BF16)
            wsc = wp.tile([128, 512], BF16)   # dep-free warmup scratch
            nc.vector.memset(wsc[:], 0.125)
            # y0+w1 first: init matmuls need only these; w2/w31 are not
            # read until the first step's L2/W31 (~2us later)
            nc.sync.dma_start(y0s[:], y0_d.ap())
            nc.sync.dma_start(w1[:], w1_d.ap())
            nc.sync.dma_start(w2[:], w2_d.ap())
            nc.sync.dma_start(w31[:], w31_d.ap())
            if has_b1 or has_b2 or has_b3:
                ones = wp.tile([1, FD], BF16)
                nc.vector.memset(ones[:], 1.0)
            if has_b1:
                b1r = wp.tile([1, 2, 128], BF16)
                nc.sync.dma_start(b1r[:], b1_d.ap())
            if has_b2:
                b2r = wp.tile([1, 2, 128], BF16)
                nc.sync.dma_start(b2r[:], b2_d.ap())
            if has_b3:
                c1r = wp.tile([1, 2, 128], BF16)
                nc.sync.dma_start(c1r[:], c1_d.ap())

            us = [up.tile([128, 2, FD], F32, tag=f"u{s}", name=f"u{s}")
                  for s in range(NS)]
            ps2s = [pp.tile([128, 2, FD], F32, tag=f"ps2{s}", name=f"ps2{s}")
                    for s in range(NS)]

            # HAM p-state warmup on scratch (no DMA deps -> overlaps input
            # DMAs); ps2 banks are rewritten fresh each step (start=True)
            for w in range(nwarm):
                nc.tensor.matmul(
                    ps2s[w % NS][:, :, :].rearrange("p a b -> p (a b)"),
                    wsc[:, 0:128], wsc[:, 0:256], start=True, stop=True,
                    skip_group_check=True)

            # init u' = a*(W1^T y0) (+ a*b1); first matmul per bank starts
            for s in range(NS):
                cs = slice(OFF[s], OFF[s] + FDS[s])
                for mc in range(2):
                    nc.tensor.matmul(us[s][:, mc, 0:FDS[s]], w1[:, mc, :],
                                     y0s[:, cs], start=(mc == 0),
                                     stop=(mc == 1) and not has_b1,
                                     skip_group_check=True)
                    if has_b1:
                        nc.tensor.matmul(us[s][:, mc, 0:FDS[s]],
                                         b1r[:, mc, :], ones[:, 0:FDS[s]],
                                         start=False, stop=(mc == 1),
                                         skip_group_check=True)

            out_ap = out_d.ap()

            for i in range(nsteps):
                h2a = stp.tile([128, 2, BL], FP8, tag="h2all", name="h2all")
                order = [(i - so) % NS for so in range(NS)]
                h1s = {}
                for s in order:
                    fd = FDS[s]
                    # ---- h1' = fp8(relu(u' * b/a)) on ACT ----
                    h1s[s] = hp.tile([128, 2, fd], FP8, tag=f"h1s{s}",
                                     name="h1")
                    nc.scalar.activation(h1s[s][:], us[s][:, :, 0:fd], RELU,
                                         scale=h1scale)
                for s in order:
                    fd = FDS[s]
                    # ---- layer 2: one DoubleRow matmul per mc ----
                    for mc in range(2):
                        nc.tensor.matmul(ps2s[s][:, mc, 0:fd],
                                         w2[:, :, mc, :], h1s[s][:],
                                         start=True, stop=not has_b2,
                                         perf_mode=DR, skip_group_check=True)
                        if has_b2:
                            nc.tensor.matmul(ps2s[s][:, mc, 0:fd],
                                             b2r[:, mc, :], ones[:, 0:fd],
                                             start=False, stop=True,
                                             skip_group_check=True)
                for s in order:
                    fd = FDS[s]
                    cs = slice(OFF[s], OFF[s] + fd)
                    # ---- h2' = fp8(max(ps2' * d/(s2 b), 0)) on DVE ----
                    nc.vector.tensor_scalar(h2a[:, :, cs],
                                            ps2s[s][:, :, 0:fd],
                                            h2scale, 0.0, op0=mult, op1=mx)
                if i < 4:
                    # scratch matmuls into the (dead) ps2 banks: raise
                    # early-step PE duty to ~97% so the free-running HAM
                    # window fires K=8/8 by ~14us worst-case regardless of
                    # phase (else up to 18 cold 2287ns steps). ps2 is dead
                    # here: DVE consumed it, next L2 re-starts the bank.
                    for w in range(4):
                        nc.tensor.matmul(ps2s[order[w]][:, 0, :],
                                         wsc[:, 0:128], wsc[:, 0:128],
                                         start=True, stop=True,
                                         skip_group_check=True)
                for s in order:
                    fd = FDS[s]
                    cs = slice(OFF[s], OFF[s] + fd)
                    # ---- u' += W31q^T h2' (DR accumulate) ----
                    for mc in range(2):
                        nc.tensor.matmul(us[s][:, mc, 0:fd],
                                         w31[:, :, mc, :],
                                         h2a[:, :, cs], start=False,
                                         stop=(mc == 1) and not has_b3,
                                         perf_mode=DR, skip_group_check=True)
                        if has_b3:
                            nc.tensor.matmul(us[s][:, mc, 0:fd],
                                             c1r[:, mc, :], ones[:, 0:fd],
                                             start=False, stop=(mc == 1),
                                             skip_group_check=True)
                # ship h2' to HBM; host divides by d and projects via dt*W3
                nc.sync.dma_start(out_ap[i], h2a[:])
    nc.compile()
    return nc


def _prep_inputs(y0, t, W1, b1, W2, b2, W3, b3):
    import ml_dtypes
    bf16 = ml_dtypes.bfloat16
    e4 = ml_dtypes.float8_e4m3fn
    dt64 = np.float64(t[1]) - np.float64(t[0])
    W31 = (dt64 * (W3.astype(np.float64) @ W1.astype(np.float64))).astype(
        np.float32)
    c1f = (dt64 * (b3.astype(np.float64) @ W1.astype(np.float64))).astype(
        np.float32)
    W3d = (dt64 * W3.astype(np.float64)).astype(np.float32)
    b3d = (dt64 * b3.astype(np.float64)).astype(np.float32)

    s2 = 240.0 / max(float(np.abs(W2).max()), 1e-30)
    s31 = 240.0 / max(float(np.abs(W31).max()), 1e-30)
    bsc = 224.0 / H1AMAX_EST
    dsc = 224.0 / H2AMAX_EST
    a = s31 * dsc

    def q8(x, s):
        return np.clip(x.astype(np.float64) * s, -240, 240).astype(
            np.float32).astype(e4)

    w2q = np.ascontiguousarray(
        q8(W2, s2).reshape(2, 128, 2, 128).transpose(1, 0, 2, 3))
    w31q = np.ascontiguousarray(
        q8(W31, s31).reshape(2, 128, 2, 128).transpose(1, 0, 2, 3))
    w1a = np.ascontiguousarray(
        (a * W1.astype(np.float64)).astype(np.float32).reshape(
            D, 2, 128)).astype(bf16)
    b1r = np.ascontiguousarray(
        (a * b1.astype(np.float64)).astype(np.float32).reshape(
            1, 2, 128)).astype(bf16)
    b2r = np.ascontiguousarray(
        (s2 * bsc * b2.astype(np.float64)).astype(np.float32).reshape(
            1, 2, 128)).astype(bf16)
    c1r = np.ascontiguousarray(
        (a * c1f.astype(np.float64)).astype(np.float32).reshape(
            1, 2, 128)).astype(bf16)

    in_maps = []
    for c in range(NCORES):
        y0T = np.ascontiguousarray(y0[c * BL:(c + 1) * BL].T).astype(bf16)
        in_maps.append({"y0T": y0T, "w1a": w1a, "w2q": w2q, "w31q": w31q,
                        "b1r": b1r, "b2r": b2r, "c1r": c1r})
    scales = dict(h1scale=float(bsc / a), h2scale=float(dsc / (s2 * bsc)),
                  d=dsc)
    return in_maps, W3d, b3d, scales


def kernel(y0, t, W1, b1, W2, b2, W3, b3, nwarm: int = 16, **run_kwargs):
    import ml_dtypes
    nsteps = int(t.shape[0]) - 1
    has_b1 = bool(np.any(b1)); has_b2 = bool(np.any(b2))
    has_b3 = bool(np.any(b3))
    in_maps, W3d, b3d, sc = _prep_inputs(y0, t, W1, b1, W2, b2, W3, b3)
    key = (nsteps, nwarm, has_b1, has_b2, has_b3,
           round(sc["h1scale"], 14), round(sc["h2scale"], 14))
    if key not in _cache:
        _cache[key] = build(nsteps, sc["h1scale"], sc["h2scale"], nwarm,
                            has_b1, has_b2, has_b3)
    nc = _cache[key]
    res = run_bass_kernel_spmd(nc, in_maps, core_ids=list(range(NCORES)),
                               **run_kwargs)
    inv_d = np.float32(1.0 / sc["d"])
    parts = []
    for c in range(NCORES):
        h2 = res.results[c]["out"]        # [nsteps, 128, 2, BL] fp8
        if h2.dtype != ml_dtypes.float8_e4m3fn:
            h2 = h2.view(ml_dtypes.float8_e4m3fn)
        nst = h2.shape[0]
        hh = h2.astype(np.float32).transpose(0, 2, 1, 3).reshape(nst, H, BL)
        hh *= inv_d
        dy = np.tensordot(hh, W3d, axes=([1], [0]))       # [nsteps, BL, D]
        dy += b3d
        dy = np.ascontiguousarray(dy.transpose(1, 0, 2))  # [BL, nsteps, D]
        yb = y0[c * BL:(c + 1) * BL].astype(np.float32)
        ys = yb[:, None, :] + np.cumsum(dy, axis=1, dtype=np.float32)
        parts.append(np.concatenate([yb[:, None, :], ys], axis=1))
    return np.concatenate(parts, axis=0).astype(np.float32)
